# revision 48
# baseline (speedup 1.0000x reference)
"""GCN layer (GCNConv + per-channel PReLU) on 8 Trainium2 NeuronCores.

out = PReLU(D^-1/2 (A+I) D^-1/2 (x @ W) + bias)

Strategy (dst-sharded graph parallelism):
  * Host: convert edge list into per-core, per-(dst-window, src-chunk) padded
    edge buckets (sharding / format prep).  Self loops are appended as real
    edges.  deg/dinv come from the same bucketing pass (bincount).
  * Device, phase 1 (replicated): h2 = (x @ W) * dinv[row]  as bf16 rows in
    DRAM - the gather table.
  * Device, main pass: for each 128-edge tile, dma_gather the source rows
    (bf16, 256B each, int16 chunk-local indices), build a one-hot selection
    matrix S[e, d] = (dst_local[e] == d) on the vector engine, and matmul
    S^T @ msgs on the tensor engine accumulating per-dst-window in PSUM.
    Epilogue fuses dinv[dst] scale + bias + PReLU.
  * Host: concatenate the 8 per-core dst shards.
"""

import os
import sys
from contextlib import ExitStack

import numpy as np

try:
    import concourse.bass as bass
except ImportError:  # pragma: no cover
    sys.path.insert(0, "/opt/trn_rl_repo")
    import concourse.bass as bass

import concourse.tile as tile
from concourse import bacc, mybir
from concourse.bass_utils import run_bass_kernel_spmd

BF16 = mybir.dt.bfloat16
F32 = mybir.dt.float32
I16 = mybir.dt.int16
NP_BF16 = mybir.dt.np(BF16)

P = 128
NCORES = 8
STRIPE_W = 4   # dst windows per stripe; one PSUM bank per window (only one
               # accumulation group may be open per PSUM bank "zero region");
               # 4-window stripes + 2 PSUM bufs -> two stripes in flight
GGRP = 2       # dst windows per dma_gather group
PH1_G = 4      # phase-1 row blocks per PSUM bank
CHUNK_ROWS = 24576  # max gather-table chunk (int16 idx range; mult of 512)

MULT = mybir.AluOpType.mult
ADD = mybir.AluOpType.add
ISEQ = mybir.AluOpType.is_equal

last_results = None  # BassKernelResults of the most recent run (for profiling)


def _ceil(a, b):
    return -(-a // b)


def _plan(x, edge_index, W):
    """Host-side sharding/bucketing. Returns (meta, shared_arrays, per_core_arrays)."""
    N, IN = x.shape
    HID = W.shape[1]
    assert HID == P, HID
    assert IN % P == 0, IN
    assert N % NCORES == 0, N
    SHARD = N // NCORES
    NWIN = _ceil(SHARD, P)
    NCHUNK = max(1, _ceil(N, CHUNK_ROWS))
    CHUNK = _ceil(N, NCHUNK)
    CHUNK = _ceil(CHUNK, 1024) * 1024  # align to phase-1 super-group writes
    NCHUNK = _ceil(N, CHUNK)
    assert CHUNK <= 32767
    NPAD = _ceil(N, P) * P
    NBLK = NPAD // P
    KH = IN // P

    src = np.asarray(edge_index[0]).astype(np.int64)
    dst = np.asarray(edge_index[1]).astype(np.int64)
    deg = np.bincount(dst, minlength=N).astype(np.float64) + 1.0
    dinv = (1.0 / np.sqrt(deg)).astype(np.float32)

    # self loops as ordinary edges
    nodes = np.arange(N, dtype=np.int64)
    src = np.concatenate([src, nodes])
    dst = np.concatenate([dst, nodes])

    core = dst // SHARD
    dstl = dst - core * SHARD
    w_ = dstl // P
    dloc = (dstl % P).astype(np.float32)
    ch_ = src // CHUNK
    sloc = (src - ch_ * CHUNK).astype(np.int16)
    NB = NWIN * NCHUNK
    bucket = w_ * NCHUNK + ch_

    counts = np.zeros((NCORES, NB), np.int64)
    for c in range(NCORES):
        counts[c] = np.bincount(bucket[core == c], minlength=NB)
    Tb = _ceil(counts.max(axis=0), P)  # blocks per (w, ch) bucket; shared schedule

    stripes = [list(range(i, min(i + STRIPE_W, NWIN))) for i in range(0, NWIN, STRIPE_W)]

    # first/last non-empty chunk per window (for matmul start/stop flags),
    # in the stripe's rotated chunk order
    first_ch = np.full(NWIN, -1, np.int64)
    last_ch = np.full(NWIN, -1, np.int64)
    for w in range(NWIN):
        si = w // STRIPE_W
        order = [(si + k) % NCHUNK for k in range(NCHUNK)]
        nz = [c_ for c_ in order if Tb[w * NCHUNK + c_] > 0]
        assert nz  # every window holds at least its self loops
        first_ch[w] = nz[0]
        last_ch[w] = nz[-1]

    # assign slot bases in schedule order and collect gather groups.
    # chunk order rotates per stripe so not every stripe's LAST chunk is the
    # one whose table is written at the very end of phase 1.
    base = np.full(NB, -1, np.int64)
    tot = 0
    groups = []  # dicts: stripe, ch, blk_base, nblk, mm=[(w, local_blk, start, stop)]
    for si, st in enumerate(stripes):
        chunk_order = [(si + k) % NCHUNK for k in range(NCHUNK)]
        for c_ in chunk_order:
            for g0 in range(0, len(st), GGRP):
                ws = st[g0:g0 + GGRP]
                mm = []
                blk_base = tot // P
                nb = 0
                for w in ws:
                    tb = int(Tb[w * NCHUNK + c_])
                    if tb == 0:
                        continue
                    base[w * NCHUNK + c_] = tot
                    for t in range(tb):
                        mm.append((w, nb + t,
                                   c_ == first_ch[w] and t == 0,
                                   c_ == last_ch[w] and t == tb - 1))
                    nb += tb
                    tot += tb * P
                if nb:
                    groups.append(dict(stripe=si, ch=c_, blk_base=blk_base,
                                       nblk=nb, mm=mm))
    TOTSLOT = tot
    TOTBLK = tot // P

    # per-core edge arrays
    idx_list, dval_list, ddst_list = [], [], []
    for c in range(NCORES):
        m = core == c
        b_c = bucket[m]
        s_c = sloc[m]
        d_c = dloc[m]
        order = np.argsort(b_c, kind="stable")
        b_s = b_c[order]
        cnt = counts[c]
        cum = np.concatenate([[0], np.cumsum(cnt)[:-1]])
        rank = np.arange(b_s.size) - cum[b_s]
        slot = base[b_s] + rank
        sidx = np.zeros(TOTSLOT, np.int16)
        dval = np.full(TOTSLOT, -1.0, np.float32)
        sidx[slot] = s_c[order]
        dval[slot] = d_c[order]
        idx_list.append(np.ascontiguousarray(
            np.tile(sidx.reshape(-1, 16).T, (8, 1))))          # [128, TOTSLOT//16] i16
        dval_list.append(np.ascontiguousarray(
            dval.reshape(-1, P).T.astype(NP_BF16)))            # [128, TOTBLK] bf16
        dvp = np.ones(NWIN * P, np.float32)
        dvp[:SHARD] = dinv[c * SHARD:(c + 1) * SHARD]
        ddst_list.append(np.ascontiguousarray(dvp.reshape(NWIN, P).T))  # [128, NWIN]

    dinv_pad = np.ones(NPAD, np.float32)
    dinv_pad[:N] = dinv
    dinv_pm = np.ascontiguousarray(dinv_pad.reshape(NBLK, P).T)  # [128, NBLK]

    xt = np.zeros((IN, NPAD), NP_BF16)
    xt[:, :N] = np.asarray(x, np.float32).T.astype(NP_BF16)

    meta = dict(N=N, IN=IN, HID=HID, SHARD=SHARD, NWIN=NWIN, NCHUNK=NCHUNK,
                CHUNK=CHUNK, NPAD=NPAD, NBLK=NBLK, KH=KH, TOTSLOT=TOTSLOT,
                TOTBLK=TOTBLK, stripes=stripes, groups=groups)
    shared = dict(xt=xt, dinv_pm=dinv_pm)
    per_core = dict(idx16=idx_list, dval=dval_list, dinv_dst=ddst_list)
    return meta, shared, per_core


def _bcast_inner(ap, n_outer, n_inner):
    """AP view [P, n_outer, n_inner] of a [P, >=n_outer] AP, broadcast on inner dim."""
    return ap.to_broadcast([P, n_outer, n_inner])


def _bcast_outer(ap, n_outer, n_inner):
    """AP view [P, n_outer, n_inner] of a [P, n_inner] AP, broadcast on outer dim."""
    return bass.AP(tensor=ap.tensor, offset=ap.offset,
                   ap=[list(ap.ap[0]), [0, n_outer], list(ap.ap[1])])


def _build(meta):
    """Build the SPMD bass program (shared by all 8 cores)."""
    N, IN, HID = meta["N"], meta["IN"], meta["HID"]
    NWIN, NCHUNK, CHUNK = meta["NWIN"], meta["NCHUNK"], meta["CHUNK"]
    NPAD, NBLK, KH = meta["NPAD"], meta["NBLK"], meta["KH"]
    TOTSLOT, TOTBLK = meta["TOTSLOT"], meta["TOTBLK"]
    stripes, groups = meta["stripes"], meta["groups"]

    nc = bacc.Bacc("TRN2", target_bir_lowering=False, debug=False,
                   num_devices=NCORES, num_swdge_queues=4)

    xt = nc.dram_tensor("xt", [IN, NPAD], BF16, kind="ExternalInput").ap()
    w2 = nc.dram_tensor("w2", [IN, HID], BF16, kind="ExternalInput").ap()
    bias2 = nc.dram_tensor("bias2", [1, HID], F32, kind="ExternalInput").ap()
    alpha2 = nc.dram_tensor("alpha2", [1, HID], F32, kind="ExternalInput").ap()
    iota2 = nc.dram_tensor("iota2", [1, P], BF16, kind="ExternalInput").ap()
    dpm = nc.dram_tensor("dpm", [P, NBLK], F32, kind="ExternalInput").ap()
    ddst = nc.dram_tensor("ddst", [P, NWIN], F32, kind="ExternalInput").ap()
    idx16 = nc.dram_tensor("idx16", [P, TOTSLOT // 16], I16, kind="ExternalInput").ap()
    dval = nc.dram_tensor("dval", [P, TOTBLK], BF16, kind="ExternalInput").ap()

    # one DRAM tensor per gather-table chunk: the HW dma_gather path mishandles
    # large base offsets on the source AP, so each chunk must start at offset 0
    h2c = [nc.dram_tensor(f"h2_{c}", [CHUNK, HID], BF16).ap()
           for c in range(NCHUNK)]
    outp = nc.dram_tensor("outp", [NWIN * P, HID], F32, kind="ExternalOutput").ap()

    h2cr = [h.rearrange("(b p) c -> p b c", p=P) for h in h2c]
    outr = outp.rearrange("(b p) c -> p b c", p=P)
    CBLK = CHUNK // P  # 128-row blocks per chunk

    with ExitStack() as ctx:
        tc = ctx.enter_context(tile.TileContext(nc))
        consts = ctx.enter_context(tc.tile_pool(name="consts", bufs=1))

        # --- resident constants -------------------------------------------
        w_sb = consts.tile([P, KH, HID], BF16)
        for kk in range(KH):
            nc.sync.dma_start(out=w_sb[:, kk, :], in_=w2[kk * P:(kk + 1) * P, :])
        iota_sb = consts.tile([P, P], BF16)
        nc.sync.dma_start(out=iota_sb[:], in_=bass.AP(
            tensor=iota2.tensor, offset=iota2.offset, ap=[[0, P], [1, P]]))
        bias_sb = consts.tile([P, HID], F32)
        nc.sync.dma_start(out=bias_sb[:], in_=bass.AP(
            tensor=bias2.tensor, offset=bias2.offset, ap=[[0, P], [1, HID]]))
        alpha_sb = consts.tile([P, HID], F32)
        nc.sync.dma_start(out=alpha_sb[:], in_=bass.AP(
            tensor=alpha2.tensor, offset=alpha2.offset, ap=[[0, P], [1, HID]]))
        dpm_sb = consts.tile([P, NBLK], F32)
        nc.sync.dma_start(out=dpm_sb[:], in_=dpm[:, :])
        ddst_sb = consts.tile([P, NWIN], F32)
        nc.sync.dma_start(out=ddst_sb[:], in_=ddst[:, :])
        dval_sb = consts.tile([P, TOTBLK], BF16)
        nc.sync.dma_start(out=dval_sb[:], in_=dval[:, :])

        # Main-pass SBUF pools are opened BEFORE the phase-1 pools so their
        # tiles get fresh SBUF rather than recycling phase-1 space. (Recycling
        # makes the first gather wait on every phase-1 matmul via WAR, killing
        # phase overlap.) PSUM is still recycled - that only gates matmuls.
        max_nblk = max(g_["nblk"] for g_ in groups)
        # per-stripe slot extents for streaming idx tiles
        stripe_blk = {}
        for g_ in groups:
            b0_, b1_ = g_["blk_base"], g_["blk_base"] + g_["nblk"]
            if g_["stripe"] in stripe_blk:
                a, b = stripe_blk[g_["stripe"]]
                stripe_blk[g_["stripe"]] = (min(a, b0_), max(b, b1_))
            else:
                stripe_blk[g_["stripe"]] = (b0_, b1_)
        max_stripe_blk = max(b - a for a, b in stripe_blk.values())
        mmsg = ctx.enter_context(tc.tile_pool(name="mmsg", bufs=16))
        msel = ctx.enter_context(tc.tile_pool(name="msel", bufs=10))
        mep = ctx.enter_context(tc.tile_pool(name="mep", bufs=6))
        midx = ctx.enter_context(tc.tile_pool(name="midx", bufs=3))

        # --- phase 1: h2 = (x @ W) * dinv[row], bf16 ----------------------
        # super-groups of SG=2*PH1_G row blocks: one wide x load, two PSUM
        # tiles, one combined h2 write; DMAs alternate between the two HWDGE
        # rings (sync / scalar) to parallelize descriptor generation
        SG = 2 * PH1_G
        with tc.tile_pool(name="p1x", bufs=3) as p1x, \
             tc.tile_pool(name="p1h", bufs=3) as p1h, \
             tc.tile_pool(name="p1ps", bufs=4, space="PSUM") as p1ps:
            nG1 = _ceil(NBLK, SG)
            for j in range(nG1):
                g = min(SG, NBLK - j * SG)
                col0 = j * SG * P
                eng = nc.sync if j % 2 == 0 else nc.scalar
                xp = p1x.tile([P, KH, SG * P], BF16, tag="xp")
                xt3 = bass.AP(
                    tensor=xt.tensor, offset=xt.offset + col0,
                    ap=[[NPAD, P], [P * NPAD, KH], [1, g * P]])
                eng.dma_start(out=xp[:, 0:KH, 0:g * P], in_=xt3)
                h2t = p1h.tile([P, SG, P], BF16, tag="h2t")
                for half in range(_ceil(g, PH1_G)):
                    gh = min(PH1_G, g - half * PH1_G)
                    ps = p1ps.tile([P, PH1_G * P], F32, tag="ps", name="ps")
                    for k in range(gh):
                        kb = half * PH1_G + k
                        for kk in range(KH):
                            nc.tensor.matmul(out=ps[:, k * P:(k + 1) * P],
                                             lhsT=xp[:, kk, kb * P:(kb + 1) * P],
                                             rhs=w_sb[:, kk, :],
                                             start=(kk == 0), stop=(kk == KH - 1))
                    ps3 = ps[:].rearrange("p (g q) -> p g q", q=P)[:, 0:gh, :]
                    b0 = j * SG + half * PH1_G
                    if (2 * j + half) % 3 != 2:
                        nc.vector.tensor_tensor(
                            out=h2t[:, half * PH1_G:half * PH1_G + gh, :], in0=ps3,
                            in1=_bcast_inner(dpm_sb[:, b0:b0 + gh], gh, P),
                            op=MULT)
                    else:
                        # spread psum evacuation over the scalar engine too
                        for k in range(gh):
                            nc.scalar.activation(
                                out=h2t[:, half * PH1_G + k, :],
                                in_=ps[:, k * P:(k + 1) * P],
                                func=mybir.ActivationFunctionType.Copy,
                                scale=dpm_sb[:, b0 + k:b0 + k + 1])
                blk0 = j * SG
                ci, cb = blk0 // CBLK, blk0 % CBLK
                eng.dma_start(out=h2cr[ci][:, cb:cb + g, :], in_=h2t[:, 0:g, :])

        # --- main pass ----------------------------------------------------
        with tc.tile_pool(name="mpsum", bufs=2, space="PSUM") as mpsum:

            pst = None
            cur_stripe = -1
            gq = 0  # rotate gathers over the 4 SWDGE queues: each queue runs
                    # on its own Q7 core pair, so desc-gen parallelizes 4x

            def epilogue(si):
                st = stripes[si]
                w0 = st[0]
                for k, w in enumerate(st):
                    pt = pst[k]
                    tmp = mep.tile([P, P], F32, tag="tmp")
                    nc.vector.scalar_tensor_tensor(
                        out=tmp[:], in0=pt[:],
                        scalar=ddst_sb[:, w:w + 1], in1=bias_sb[:],
                        op0=MULT, op1=ADD)
                    ot = mep.tile([P, P], F32, tag="ot")
                    if meta.get("alpha_01", False):
                        # PReLU(x) == max(x, alpha*x) when 0 <= alpha <= 1
                        mn = mep.tile([P, P], F32, tag="mn")
                        nc.vector.tensor_tensor(out=mn[:], in0=tmp[:],
                                                in1=alpha_sb[:], op=MULT)
                        nc.vector.tensor_tensor(out=ot[:], in0=tmp[:], in1=mn[:],
                                                op=mybir.AluOpType.max)
                    else:
                        mx = mep.tile([P, P], F32, tag="mx")
                        nc.vector.tensor_scalar_max(mx[:], tmp[:], 0.0)
                        mn = mep.tile([P, P], F32, tag="mn")
                        nc.vector.tensor_scalar_min(mn[:], tmp[:], 0.0)
                        nc.vector.tensor_tensor(out=mn[:], in0=mn[:],
                                                in1=alpha_sb[:], op=MULT)
                        nc.vector.tensor_tensor(out=ot[:], in0=mx[:], in1=mn[:],
                                                op=ADD)
                    nc.sync.dma_start(out=outr[:, w:w + 1, :],
                                      in_=ot[:].rearrange("p (g q) -> p g q", q=P))

            sidx = None
            sblk0 = 0
            for grp in groups:
                if grp["stripe"] != cur_stripe:
                    if cur_stripe >= 0:
                        epilogue(cur_stripe)
                    cur_stripe = grp["stripe"]
                    pst = [mpsum.tile([P, P], F32, tag=f"ps{i}", name=f"pst{i}")
                           for i in range(len(stripes[cur_stripe]))]
                    sblk0, sblk1 = stripe_blk[cur_stripe]
                    sidx = midx.tile([P, max_stripe_blk * 8], I16, tag="sidx")
                    nc.scalar.dma_start(
                        out=sidx[:, 0:(sblk1 - sblk0) * 8],
                        in_=idx16[:, sblk0 * 8:sblk1 * 8])
                nb = grp["nblk"]
                c_ = grp["ch"]
                b0 = grp["blk_base"]
                mt = mmsg.tile([P, max_nblk, P], BF16, tag="mt")
                nc.gpsimd.dma_gather(
                    mt[:, 0:nb, :],
                    h2c[c_][:, :],
                    sidx[:, (b0 - sblk0) * 8:(b0 - sblk0 + nb) * 8],
                    nb * P,
                    nb * P,
                    P,
                    single_packet=False,
                    queue_num=gq % 4,
                )
                gq += 1
                st_ = msel.tile([P, max_nblk, P], BF16, tag="st")
                nc.vector.tensor_tensor(
                    out=st_[:, 0:nb, :],
                    in0=_bcast_inner(dval_sb[:, b0:b0 + nb], nb, P),
                    in1=_bcast_outer(iota_sb[:], nb, P),
                    op=ISEQ)
                w0 = stripes[cur_stripe][0]
                for (w, b, fl_start, fl_stop) in grp["mm"]:
                    nc.tensor.matmul(out=pst[w - w0][:],
                                     lhsT=st_[:, b, :], rhs=mt[:, b, :],
                                     start=fl_start, stop=fl_stop)
            epilogue(cur_stripe)

    _hoist_reg_moves(nc)
    return nc


def _hoist_reg_moves(nc):
    """Tile defers constant reg-writes (to_reg) and does not track the
    register dependency of custom instructions like InstDMAGatherAnt, so the
    defining InstRegisterMove can land after its use. Hoist each such move to
    just before the first use of its register within the block (pure constant
    write on the same engine - always safe)."""
    for bb in nc.m.functions[0].blocks:
        insts = bb.instructions
        use_pos = {}
        movs = []
        for i, ins in enumerate(insts):
            for a in ins.ins:
                if isinstance(a, mybir.RegisterAccess):
                    use_pos.setdefault(a.regref, i)
            if isinstance(ins, mybir.InstRegisterMove):
                outs = list(ins.outs)
                if outs and isinstance(outs[0], mybir.RegisterAccess):
                    movs.append((i, outs[0].regref, ins))
        for i, regref, ins in sorted(movs, reverse=True):
            first_use = use_pos.get(regref)
            if first_use is not None and first_use < i:
                del insts[i]
                insts.insert(first_use, ins)


def kernel(x, edge_index, W, bias, alpha):
    global last_results
    x = np.asarray(x)
    edge_index = np.asarray(edge_index)
    W = np.asarray(W)
    bias = np.asarray(bias, dtype=np.float32)
    alpha = np.asarray(alpha, dtype=np.float32)

    meta, shared, per_core = _plan(x, edge_index, W)
    meta["alpha_01"] = bool(np.all((alpha >= 0.0) & (alpha <= 1.0)))
    nc = _build(meta)
    if not nc.is_finalized():
        nc.finalize()  # Bacc: runs compile() passes (reg alloc, wait splitting)

    w2 = np.ascontiguousarray(np.asarray(W, np.float32).astype(NP_BF16))
    bias2 = bias.reshape(1, -1)
    alpha2 = alpha.reshape(1, -1)
    iota2 = np.arange(P, dtype=np.float32).astype(NP_BF16).reshape(1, P)

    in_maps = []
    for c in range(NCORES):
        in_maps.append(dict(
            xt=shared["xt"], w2=w2, bias2=bias2, alpha2=alpha2, iota2=iota2,
            dpm=shared["dinv_pm"], ddst=per_core["dinv_dst"][c],
            idx16=per_core["idx16"][c], dval=per_core["dval"][c],
        ))

    res = run_bass_kernel_spmd(nc, in_maps, core_ids=list(range(NCORES)))
    last_results = res
    SHARD = meta["SHARD"]
    out = np.concatenate([res.results[c]["outp"][:SHARD] for c in range(NCORES)],
                         axis=0)
    return out.astype(np.float32)


# revision 49
# speedup vs baseline: 1.4201x; 1.4201x over previous
"""GCN layer (GCNConv + per-channel PReLU) on 8 Trainium2 NeuronCores.

out = PReLU(D^-1/2 (A+I) D^-1/2 (x @ W) + bias)

Strategy (dst-sharded graph parallelism):
  * Host: convert edge list into per-core, per-(dst-window, src-chunk) padded
    edge buckets (sharding / format prep).  Self loops are appended as real
    edges.  deg/dinv come from the same bucketing pass (bincount).
  * Device, phase 1 (replicated): h2 = (x @ W) * dinv[row]  as bf16 rows in
    DRAM - the gather table.
  * Device, main pass: for each 128-edge tile, dma_gather the source rows
    (bf16, 256B each, int16 chunk-local indices), build a one-hot selection
    matrix S[e, d] = (dst_local[e] == d) on the vector engine, and matmul
    S^T @ msgs on the tensor engine accumulating per-dst-window in PSUM.
    Epilogue fuses dinv[dst] scale + bias + PReLU.
  * Host: concatenate the 8 per-core dst shards.
"""

import os
import sys
from contextlib import ExitStack

import numpy as np

try:
    import concourse.bass as bass
except ImportError:  # pragma: no cover
    sys.path.insert(0, "/opt/trn_rl_repo")
    import concourse.bass as bass

import concourse.tile as tile
from concourse import bacc, mybir
from concourse.bass_utils import run_bass_kernel_spmd

BF16 = mybir.dt.bfloat16
F32 = mybir.dt.float32
I16 = mybir.dt.int16
NP_BF16 = mybir.dt.np(BF16)

P = 128
NCORES = 8
STRIPE_W = 4   # dst windows per stripe; one PSUM bank per window (only one
               # accumulation group may be open per PSUM bank "zero region");
               # 4-window stripes + 2 PSUM bufs -> two stripes in flight
GGRP = 4       # dst windows per dma_gather group
PH1_G = 4      # phase-1 row blocks per PSUM bank
CHUNK_ROWS = 24576  # max gather-table chunk (int16 idx range; mult of 512)

MULT = mybir.AluOpType.mult
ADD = mybir.AluOpType.add
ISEQ = mybir.AluOpType.is_equal

last_results = None  # BassKernelResults of the most recent run (for profiling)


def _ceil(a, b):
    return -(-a // b)


def _plan(x, edge_index, W):
    """Host-side sharding/bucketing. Returns (meta, shared_arrays, per_core_arrays)."""
    N, IN = x.shape
    HID = W.shape[1]
    assert HID == P, HID
    assert IN % P == 0, IN
    assert N % NCORES == 0, N
    SHARD = N // NCORES
    NWIN = _ceil(SHARD, P)
    NCHUNK = max(1, _ceil(N, CHUNK_ROWS))
    CHUNK = _ceil(N, NCHUNK)
    CHUNK = _ceil(CHUNK, 1024) * 1024  # align to phase-1 super-group writes
    NCHUNK = _ceil(N, CHUNK)
    assert CHUNK <= 32767
    NPAD = _ceil(N, P) * P
    NBLK = NPAD // P
    KH = IN // P

    src = np.asarray(edge_index[0]).astype(np.int64)
    dst = np.asarray(edge_index[1]).astype(np.int64)
    deg = np.bincount(dst, minlength=N).astype(np.float64) + 1.0
    dinv = (1.0 / np.sqrt(deg)).astype(np.float32)

    # self loops as ordinary edges
    nodes = np.arange(N, dtype=np.int64)
    src = np.concatenate([src, nodes])
    dst = np.concatenate([dst, nodes])

    core = dst // SHARD
    dstl = dst - core * SHARD
    w_ = dstl // P
    dloc = (dstl % P).astype(np.float32)
    ch_ = src // CHUNK
    sloc = (src - ch_ * CHUNK).astype(np.int16)
    NB = NWIN * NCHUNK
    bucket = w_ * NCHUNK + ch_

    counts = np.zeros((NCORES, NB), np.int64)
    for c in range(NCORES):
        counts[c] = np.bincount(bucket[core == c], minlength=NB)
    Tb = _ceil(counts.max(axis=0), P)  # blocks per (w, ch) bucket; shared schedule

    stripes = [list(range(i, min(i + STRIPE_W, NWIN))) for i in range(0, NWIN, STRIPE_W)]

    # first/last non-empty chunk per window (for matmul start/stop flags),
    # in the stripe's rotated chunk order
    first_ch = np.full(NWIN, -1, np.int64)
    last_ch = np.full(NWIN, -1, np.int64)
    for w in range(NWIN):
        si = w // STRIPE_W
        order = [(si + k) % NCHUNK for k in range(NCHUNK)]
        nz = [c_ for c_ in order if Tb[w * NCHUNK + c_] > 0]
        assert nz  # every window holds at least its self loops
        first_ch[w] = nz[0]
        last_ch[w] = nz[-1]

    # assign slot bases in schedule order and collect gather groups.
    # chunk order rotates per stripe so not every stripe's LAST chunk is the
    # one whose table is written at the very end of phase 1.
    base = np.full(NB, -1, np.int64)
    tot = 0
    groups = []  # dicts: stripe, ch, blk_base, nblk, mm=[(w, local_blk, start, stop)]
    for si, st in enumerate(stripes):
        chunk_order = [(si + k) % NCHUNK for k in range(NCHUNK)]
        for c_ in chunk_order:
            for g0 in range(0, len(st), GGRP):
                ws = st[g0:g0 + GGRP]
                mm = []
                blk_base = tot // P
                nb = 0
                for w in ws:
                    tb = int(Tb[w * NCHUNK + c_])
                    if tb == 0:
                        continue
                    base[w * NCHUNK + c_] = tot
                    for t in range(tb):
                        mm.append((w, nb + t,
                                   c_ == first_ch[w] and t == 0,
                                   c_ == last_ch[w] and t == tb - 1))
                    nb += tb
                    tot += tb * P
                if nb:
                    groups.append(dict(stripe=si, ch=c_, blk_base=blk_base,
                                       nblk=nb, mm=mm))
    TOTSLOT = tot
    TOTBLK = tot // P

    # per-core edge arrays
    idx_list, dval_list, ddst_list = [], [], []
    for c in range(NCORES):
        m = core == c
        b_c = bucket[m]
        s_c = sloc[m]
        d_c = dloc[m]
        order = np.argsort(b_c, kind="stable")
        b_s = b_c[order]
        cnt = counts[c]
        cum = np.concatenate([[0], np.cumsum(cnt)[:-1]])
        rank = np.arange(b_s.size) - cum[b_s]
        slot = base[b_s] + rank
        sidx = np.zeros(TOTSLOT, np.int16)
        dval = np.full(TOTSLOT, -1.0, np.float32)
        sidx[slot] = s_c[order]
        dval[slot] = d_c[order]
        idx_list.append(np.ascontiguousarray(
            np.tile(sidx.reshape(-1, 16).T, (8, 1))))          # [128, TOTSLOT//16] i16
        dval_list.append(np.ascontiguousarray(
            dval.reshape(-1, P).T.astype(NP_BF16)))            # [128, TOTBLK] bf16
        dvp = np.ones(NWIN * P, np.float32)
        dvp[:SHARD] = dinv[c * SHARD:(c + 1) * SHARD]
        ddst_list.append(np.ascontiguousarray(dvp.reshape(NWIN, P).T))  # [128, NWIN]

    dinv_pad = np.ones(NPAD, np.float32)
    dinv_pad[:N] = dinv
    dinv_pm = np.ascontiguousarray(dinv_pad.reshape(NBLK, P).T)  # [128, NBLK]

    xt = np.zeros((IN, NPAD), NP_BF16)
    xt[:, :N] = np.asarray(x, np.float32).T.astype(NP_BF16)

    meta = dict(N=N, IN=IN, HID=HID, SHARD=SHARD, NWIN=NWIN, NCHUNK=NCHUNK,
                CHUNK=CHUNK, NPAD=NPAD, NBLK=NBLK, KH=KH, TOTSLOT=TOTSLOT,
                TOTBLK=TOTBLK, stripes=stripes, groups=groups)
    shared = dict(xt=xt, dinv_pm=dinv_pm)
    per_core = dict(idx16=idx_list, dval=dval_list, dinv_dst=ddst_list)
    return meta, shared, per_core


def _bcast_inner(ap, n_outer, n_inner):
    """AP view [P, n_outer, n_inner] of a [P, >=n_outer] AP, broadcast on inner dim."""
    return ap.to_broadcast([P, n_outer, n_inner])


def _bcast_outer(ap, n_outer, n_inner):
    """AP view [P, n_outer, n_inner] of a [P, n_inner] AP, broadcast on outer dim."""
    return bass.AP(tensor=ap.tensor, offset=ap.offset,
                   ap=[list(ap.ap[0]), [0, n_outer], list(ap.ap[1])])


def _build(meta):
    """Build the SPMD bass program (shared by all 8 cores)."""
    N, IN, HID = meta["N"], meta["IN"], meta["HID"]
    NWIN, NCHUNK, CHUNK = meta["NWIN"], meta["NCHUNK"], meta["CHUNK"]
    NPAD, NBLK, KH = meta["NPAD"], meta["NBLK"], meta["KH"]
    TOTSLOT, TOTBLK = meta["TOTSLOT"], meta["TOTBLK"]
    stripes, groups = meta["stripes"], meta["groups"]

    nc = bacc.Bacc("TRN2", target_bir_lowering=False, debug=False,
                   num_devices=NCORES, num_swdge_queues=4)

    xt = nc.dram_tensor("xt", [IN, NPAD], BF16, kind="ExternalInput").ap()
    w2 = nc.dram_tensor("w2", [IN, HID], BF16, kind="ExternalInput").ap()
    bias2 = nc.dram_tensor("bias2", [1, HID], F32, kind="ExternalInput").ap()
    alpha2 = nc.dram_tensor("alpha2", [1, HID], F32, kind="ExternalInput").ap()
    iota2 = nc.dram_tensor("iota2", [1, P], BF16, kind="ExternalInput").ap()
    dpm = nc.dram_tensor("dpm", [P, NBLK], F32, kind="ExternalInput").ap()
    ddst = nc.dram_tensor("ddst", [P, NWIN], F32, kind="ExternalInput").ap()
    idx16 = nc.dram_tensor("idx16", [P, TOTSLOT // 16], I16, kind="ExternalInput").ap()
    dval = nc.dram_tensor("dval", [P, TOTBLK], BF16, kind="ExternalInput").ap()

    # one DRAM tensor per gather-table chunk: the HW dma_gather path mishandles
    # large base offsets on the source AP, so each chunk must start at offset 0
    h2c = [nc.dram_tensor(f"h2_{c}", [CHUNK, HID], BF16).ap()
           for c in range(NCHUNK)]
    outp = nc.dram_tensor("outp", [NWIN * P, HID], F32, kind="ExternalOutput").ap()

    h2cr = [h.rearrange("(b p) c -> p b c", p=P) for h in h2c]
    outr = outp.rearrange("(b p) c -> p b c", p=P)
    CBLK = CHUNK // P  # 128-row blocks per chunk

    with ExitStack() as ctx:
        tc = ctx.enter_context(tile.TileContext(nc))
        consts = ctx.enter_context(tc.tile_pool(name="consts", bufs=1))

        # --- resident constants -------------------------------------------
        w_sb = consts.tile([P, KH, HID], BF16)
        for kk in range(KH):
            nc.sync.dma_start(out=w_sb[:, kk, :], in_=w2[kk * P:(kk + 1) * P, :])
        iota_sb = consts.tile([P, P], BF16)
        nc.sync.dma_start(out=iota_sb[:], in_=bass.AP(
            tensor=iota2.tensor, offset=iota2.offset, ap=[[0, P], [1, P]]))
        bias_sb = consts.tile([P, HID], F32)
        nc.sync.dma_start(out=bias_sb[:], in_=bass.AP(
            tensor=bias2.tensor, offset=bias2.offset, ap=[[0, P], [1, HID]]))
        alpha_sb = consts.tile([P, HID], F32)
        nc.sync.dma_start(out=alpha_sb[:], in_=bass.AP(
            tensor=alpha2.tensor, offset=alpha2.offset, ap=[[0, P], [1, HID]]))
        dpm_sb = consts.tile([P, NBLK], F32)
        nc.sync.dma_start(out=dpm_sb[:], in_=dpm[:, :])
        ddst_sb = consts.tile([P, NWIN], F32)
        nc.sync.dma_start(out=ddst_sb[:], in_=ddst[:, :])
        dval_sb = consts.tile([P, TOTBLK], BF16)
        nc.sync.dma_start(out=dval_sb[:], in_=dval[:, :])

        # Main-pass SBUF pools are opened BEFORE the phase-1 pools so their
        # tiles get fresh SBUF rather than recycling phase-1 space. (Recycling
        # makes the first gather wait on every phase-1 matmul via WAR, killing
        # phase overlap.) PSUM is still recycled - that only gates matmuls.
        max_nblk = max(g_["nblk"] for g_ in groups)
        # per-stripe slot extents for streaming idx tiles
        stripe_blk = {}
        for g_ in groups:
            b0_, b1_ = g_["blk_base"], g_["blk_base"] + g_["nblk"]
            if g_["stripe"] in stripe_blk:
                a, b = stripe_blk[g_["stripe"]]
                stripe_blk[g_["stripe"]] = (min(a, b0_), max(b, b1_))
            else:
                stripe_blk[g_["stripe"]] = (b0_, b1_)
        max_stripe_blk = max(b - a for a, b in stripe_blk.values())
        mmsg = ctx.enter_context(tc.tile_pool(name="mmsg", bufs=16))
        msel = ctx.enter_context(tc.tile_pool(name="msel", bufs=10))
        mep = ctx.enter_context(tc.tile_pool(name="mep", bufs=6))
        midx = ctx.enter_context(tc.tile_pool(name="midx", bufs=3))

        # --- phase 1: h2 = (x @ W) * dinv[row], bf16 ----------------------
        # super-groups of SG=2*PH1_G row blocks: one wide x load, two PSUM
        # tiles, one combined h2 write; DMAs alternate between the two HWDGE
        # rings (sync / scalar) to parallelize descriptor generation
        SG = 2 * PH1_G
        with tc.tile_pool(name="p1x", bufs=3) as p1x, \
             tc.tile_pool(name="p1h", bufs=3) as p1h, \
             tc.tile_pool(name="p1ps", bufs=4, space="PSUM") as p1ps:
            nG1 = _ceil(NBLK, SG)
            for j in range(nG1):
                g = min(SG, NBLK - j * SG)
                col0 = j * SG * P
                eng = nc.sync if j % 2 == 0 else nc.scalar
                xp = p1x.tile([P, KH, SG * P], BF16, tag="xp")
                xt3 = bass.AP(
                    tensor=xt.tensor, offset=xt.offset + col0,
                    ap=[[NPAD, P], [P * NPAD, KH], [1, g * P]])
                eng.dma_start(out=xp[:, 0:KH, 0:g * P], in_=xt3)
                h2t = p1h.tile([P, SG, P], BF16, tag="h2t")
                for half in range(_ceil(g, PH1_G)):
                    gh = min(PH1_G, g - half * PH1_G)
                    ps = p1ps.tile([P, PH1_G * P], F32, tag="ps", name="ps")
                    for k in range(gh):
                        kb = half * PH1_G + k
                        for kk in range(KH):
                            nc.tensor.matmul(out=ps[:, k * P:(k + 1) * P],
                                             lhsT=xp[:, kk, kb * P:(kb + 1) * P],
                                             rhs=w_sb[:, kk, :],
                                             start=(kk == 0), stop=(kk == KH - 1))
                    ps3 = ps[:].rearrange("p (g q) -> p g q", q=P)[:, 0:gh, :]
                    b0 = j * SG + half * PH1_G
                    if (2 * j + half) % 3 != 2:
                        nc.vector.tensor_tensor(
                            out=h2t[:, half * PH1_G:half * PH1_G + gh, :], in0=ps3,
                            in1=_bcast_inner(dpm_sb[:, b0:b0 + gh], gh, P),
                            op=MULT)
                    else:
                        # spread psum evacuation over the scalar engine too
                        for k in range(gh):
                            nc.scalar.activation(
                                out=h2t[:, half * PH1_G + k, :],
                                in_=ps[:, k * P:(k + 1) * P],
                                func=mybir.ActivationFunctionType.Copy,
                                scale=dpm_sb[:, b0 + k:b0 + k + 1])
                blk0 = j * SG
                ci, cb = blk0 // CBLK, blk0 % CBLK
                eng.dma_start(out=h2cr[ci][:, cb:cb + g, :], in_=h2t[:, 0:g, :])

        # --- main pass ----------------------------------------------------
        with tc.tile_pool(name="mpsum", bufs=2, space="PSUM") as mpsum:

            pst = None
            cur_stripe = -1
            gq = 0  # rotate gathers over the 4 SWDGE queues: each queue runs
                    # on its own Q7 core pair, so desc-gen parallelizes 4x

            def epilogue(si):
                st = stripes[si]
                w0 = st[0]
                for k, w in enumerate(st):
                    pt = pst[k]
                    tmp = mep.tile([P, P], F32, tag="tmp")
                    nc.vector.scalar_tensor_tensor(
                        out=tmp[:], in0=pt[:],
                        scalar=ddst_sb[:, w:w + 1], in1=bias_sb[:],
                        op0=MULT, op1=ADD)
                    ot = mep.tile([P, P], F32, tag="ot")
                    if meta.get("alpha_01", False):
                        # PReLU(x) == max(x, alpha*x) when 0 <= alpha <= 1
                        mn = mep.tile([P, P], F32, tag="mn")
                        nc.vector.tensor_tensor(out=mn[:], in0=tmp[:],
                                                in1=alpha_sb[:], op=MULT)
                        nc.vector.tensor_tensor(out=ot[:], in0=tmp[:], in1=mn[:],
                                                op=mybir.AluOpType.max)
                    else:
                        mx = mep.tile([P, P], F32, tag="mx")
                        nc.vector.tensor_scalar_max(mx[:], tmp[:], 0.0)
                        mn = mep.tile([P, P], F32, tag="mn")
                        nc.vector.tensor_scalar_min(mn[:], tmp[:], 0.0)
                        nc.vector.tensor_tensor(out=mn[:], in0=mn[:],
                                                in1=alpha_sb[:], op=MULT)
                        nc.vector.tensor_tensor(out=ot[:], in0=mx[:], in1=mn[:],
                                                op=ADD)
                    nc.sync.dma_start(out=outr[:, w:w + 1, :],
                                      in_=ot[:].rearrange("p (g q) -> p g q", q=P))

            sidx = None
            sblk0 = 0
            for grp in groups:
                if grp["stripe"] != cur_stripe:
                    if cur_stripe >= 0:
                        epilogue(cur_stripe)
                    cur_stripe = grp["stripe"]
                    pst = [mpsum.tile([P, P], F32, tag=f"ps{i}", name=f"pst{i}")
                           for i in range(len(stripes[cur_stripe]))]
                    sblk0, sblk1 = stripe_blk[cur_stripe]
                    sidx = midx.tile([P, max_stripe_blk * 8], I16, tag="sidx")
                    nc.scalar.dma_start(
                        out=sidx[:, 0:(sblk1 - sblk0) * 8],
                        in_=idx16[:, sblk0 * 8:sblk1 * 8])
                nb = grp["nblk"]
                c_ = grp["ch"]
                b0 = grp["blk_base"]
                mt = mmsg.tile([P, max_nblk, P], BF16, tag="mt")
                nc.gpsimd.dma_gather(
                    mt[:, 0:nb, :],
                    h2c[c_][:, :],
                    sidx[:, (b0 - sblk0) * 8:(b0 - sblk0 + nb) * 8],
                    nb * P,
                    nb * P,
                    P,
                    single_packet=False,
                    queue_num=gq % 4,
                )
                gq += 1
                st_ = msel.tile([P, max_nblk, P], BF16, tag="st")
                nc.vector.tensor_tensor(
                    out=st_[:, 0:nb, :],
                    in0=_bcast_inner(dval_sb[:, b0:b0 + nb], nb, P),
                    in1=_bcast_outer(iota_sb[:], nb, P),
                    op=ISEQ)
                w0 = stripes[cur_stripe][0]
                for (w, b, fl_start, fl_stop) in grp["mm"]:
                    nc.tensor.matmul(out=pst[w - w0][:],
                                     lhsT=st_[:, b, :], rhs=mt[:, b, :],
                                     start=fl_start, stop=fl_stop)
            epilogue(cur_stripe)

    _hoist_reg_moves(nc)
    return nc


def _hoist_reg_moves(nc):
    """Tile defers constant reg-writes (to_reg) and does not track the
    register dependency of custom instructions like InstDMAGatherAnt, so the
    defining InstRegisterMove can land after its use. Hoist each such move to
    just before the first use of its register within the block (pure constant
    write on the same engine - always safe)."""
    for bb in nc.m.functions[0].blocks:
        insts = bb.instructions
        use_pos = {}
        movs = []
        for i, ins in enumerate(insts):
            for a in ins.ins:
                if isinstance(a, mybir.RegisterAccess):
                    use_pos.setdefault(a.regref, i)
            if isinstance(ins, mybir.InstRegisterMove):
                outs = list(ins.outs)
                if outs and isinstance(outs[0], mybir.RegisterAccess):
                    movs.append((i, outs[0].regref, ins))
        for i, regref, ins in sorted(movs, reverse=True):
            first_use = use_pos.get(regref)
            if first_use is not None and first_use < i:
                del insts[i]
                insts.insert(first_use, ins)


def kernel(x, edge_index, W, bias, alpha):
    global last_results
    x = np.asarray(x)
    edge_index = np.asarray(edge_index)
    W = np.asarray(W)
    bias = np.asarray(bias, dtype=np.float32)
    alpha = np.asarray(alpha, dtype=np.float32)

    meta, shared, per_core = _plan(x, edge_index, W)
    meta["alpha_01"] = bool(np.all((alpha >= 0.0) & (alpha <= 1.0)))
    nc = _build(meta)
    if not nc.is_finalized():
        nc.finalize()  # Bacc: runs compile() passes (reg alloc, wait splitting)

    w2 = np.ascontiguousarray(np.asarray(W, np.float32).astype(NP_BF16))
    bias2 = bias.reshape(1, -1)
    alpha2 = alpha.reshape(1, -1)
    iota2 = np.arange(P, dtype=np.float32).astype(NP_BF16).reshape(1, P)

    in_maps = []
    for c in range(NCORES):
        in_maps.append(dict(
            xt=shared["xt"], w2=w2, bias2=bias2, alpha2=alpha2, iota2=iota2,
            dpm=shared["dinv_pm"], ddst=per_core["dinv_dst"][c],
            idx16=per_core["idx16"][c], dval=per_core["dval"][c],
        ))

    res = run_bass_kernel_spmd(nc, in_maps, core_ids=list(range(NCORES)))
    last_results = res
    SHARD = meta["SHARD"]
    out = np.concatenate([res.results[c]["outp"][:SHARD] for c in range(NCORES)],
                         axis=0)
    return out.astype(np.float32)
